# revision 12
# baseline (speedup 1.0000x reference)
"""Trainium2 Bass kernel for causal FFT convolution (nn_CausalConvolution).

y = irfft(rfft(bf16(x), 2T) * rfft(h, 2T))[..., :T],  x,h: (8, 64, 65536) fp32.

Identity used: with z = bf16(x) + i*h,  y = Im(iFFT(FFT_2T(z)^2)) / 2.
One complex forward + one complex inverse FFT per channel, N = 131072,
decomposed as radix (128, 128, 8) matmul stages on the PE with PE-transposes
between stages. 512 channels sharded 64-per-core across 8 NeuronCores.

v3 (from measured v1/v2 profiles): f32r datapath (PSUM-source elementwise ops
dodge the SBUF-source errata and cast penalties), x/h stacked into a single
128-partition f32r rhs for stage 1 (halves its matmuls), GpSimd absorbs the
SBUF-source adds (it was idle), transpose evacuations split scalar/vector.

Self-contained: shapes/sharding hardcoded; tables computed with numpy here.
"""
import numpy as np
import ml_dtypes
from contextlib import ExitStack

import concourse.bass as bass
import concourse.bacc as bacc
import concourse.tile as tile
import concourse.mybir as mybir
from concourse.bass_utils import run_bass_kernel_spmd

F32 = mybir.dt.float32
F32R = mybir.dt.float32r
BF16 = mybir.dt.bfloat16
MUL = mybir.AluOpType.mult
ADD = mybir.AluOpType.add
SUB = mybir.AluOpType.subtract

Bsz, Csz, T = 8, 64, 65536
NFFT = 2 * T
NCORES = 8
CPC = (Bsz * Csz) // NCORES          # 64 channels per core
NBLK = CPC // 2                      # 2 channels per block

_Wc = lambda M, E: np.exp(-2j * np.pi * E / M)


def _gen_tables():
    F128 = _Wc(128, np.outer(np.arange(128), np.arange(128)))
    F8 = _Wc(8, np.outer(np.arange(8), np.arange(8)))
    W1024bd = _Wc(1024, np.outer(np.arange(8), np.arange(128)))     # [b, d]
    TW1_cab = _Wc(NFFT, (8 * np.arange(128)[None, :, None]
                         + np.arange(8)[None, None, :])
                  * np.arange(128)[:, None, None])                  # [c, a, b]

    f32 = lambda v: np.ascontiguousarray(v, dtype=np.float32)
    t = {}
    # ---- S1 stacked stationaries: rhs rows 0-63 = bf16(x), 64-127 = h ----
    s1re = np.vstack([F128[:64].real, -F128[:64].imag])             # [128, 128]
    s1im = np.vstack([F128[:64].imag, F128[:64].real])
    t["s1"] = np.stack([f32(s1re), f32(s1im)])                      # [2,128,128]

    # ---- packed [n,128,128] f32r stationaries ----
    mats = []
    idx = {}

    def put(name, m):
        idx[name] = len(mats)
        mats.append(f32(m))

    S2 = F128[None, :, :] * W1024bd[:, None, :]                     # [b, a, d]
    for b in range(8):
        put(f"s2_re{b}", S2[b].real)
        put(f"s2_im{b}", S2[b].imag)
        put(f"s2_imn{b}", -S2[b].imag)
    S3 = np.zeros((128, 128), np.complex128)
    for b in range(8):
        for e in range(8):
            for c16 in range(16):
                S3[c16 * 8 + b, c16 * 8 + e] = F8[b, e]
    put("s3_re", S3.real)
    put("s3_im", S3.imag)
    put("s3_imn", -S3.imag)
    S3p = np.zeros((128, 128), np.complex128)
    for e in range(8):
        for bp in range(8):
            for c16 in range(16):
                S3p[c16 * 8 + e, c16 * 8 + bp] = np.conj(F8[bp, e])
    put("s3p_re", S3p.real)          # multiplies rhs_re -> psum_re
    put("s3p_im", S3p.imag)          # rhs_re -> psum_im
    put("s3p_imn2", -2 * S3p.imag)   # rhs_im -> psum_re (x2: Sim stored halved)
    put("s3p_re2", 2 * S3p.real)     # rhs_im -> psum_im
    S2p = np.conj(S2).transpose(0, 2, 1)                            # [bp, d, a]
    for b in range(8):
        put(f"s2p_re{b}", S2p[b].real)
        put(f"s2p_im{b}", S2p[b].imag)
        put(f"s2p_imn{b}", -S2p[b].imag)
    put("ident", np.eye(128))
    t["st128"] = np.stack(mats)                                     # [n,128,128]
    t["st128_idx"] = idx

    # ---- S1' (contract c, produce u<64, imag plane only, scale 1/(2N)) ----
    S1p = np.conj(F128).T[:, :64] / (2.0 * NFFT)                    # [c, u]
    t["s1p"] = np.stack([f32(S1p.real), f32(S1p.imag)])             # [2,128,64]

    # ---- twiddle tables ----
    m_ord = TW1_cab.reshape(128, 1024)
    tw1p = np.zeros((128, 2048), np.complex128)                     # [a, bp*256+ch*128+c]
    for bp in range(8):
        for ch in range(2):
            sl = slice(bp * 256 + ch * 128, bp * 256 + ch * 128 + 128)
            tw1p[:, sl] = np.conj(TW1_cab[:, :, bp]).T
    bf = lambda v: np.ascontiguousarray(v, np.float32).astype(ml_dtypes.bfloat16)
    t["tw"] = np.concatenate(
        [bf(m_ord.real), bf(m_ord.imag),
         bf(tw1p.real), bf(tw1p.imag)], axis=1)                     # [128, 6144]
    return t


def _build(n_blocks=NBLK):
    tabs = _gen_tables()
    nc = bacc.Bacc("TRN2", target_bir_lowering=False, debug=False)

    xh_d = nc.dram_tensor("xh_in", [CPC, 128, 1024], F32R, kind="ExternalInput").ap()
    nst = tabs["st128"].shape[0]
    st_d = nc.dram_tensor("st_in", [nst, 128, 128], F32R, kind="ExternalInput").ap()
    s1_d = nc.dram_tensor("s1_in", [2, 128, 128], F32R, kind="ExternalInput").ap()
    s1p_d = nc.dram_tensor("s1p_in", [2, 128, 64], F32R, kind="ExternalInput").ap()
    tw_d = nc.dram_tensor("tw_in", [128, 6144], BF16, kind="ExternalInput").ap()
    id16_d = nc.dram_tensor("id16_in", [128, 128], BF16, kind="ExternalInput").ap()
    y_d = nc.dram_tensor("y_out", [CPC, 64, 1024], F32, kind="ExternalOutput").ap()

    with tile.TileContext(nc) as tc, ExitStack() as ctx:
        const = ctx.enter_context(tc.tile_pool(name="const", bufs=1))
        data = ctx.enter_context(tc.tile_pool(name="io", bufs=2))
        stage = ctx.enter_context(tc.tile_pool(name="stage", bufs=4))
        stageB = ctx.enter_context(tc.tile_pool(name="stageB", bufs=2))
        tmp = ctx.enter_context(tc.tile_pool(name="tmp", bufs=2))
        pool16 = ctx.enter_context(tc.tile_pool(name="p16", bufs=3))
        psum = ctx.enter_context(tc.tile_pool(name="psum", bufs=2, space="PSUM"))

        # ---- load constant tables once ----
        st = const.tile([128, nst * 128], F32R, tag="st")
        nc.sync.dma_start(
            st[:].rearrange("p (n c) -> p n c", n=nst),
            st_d.rearrange("n p c -> p n c"))
        s1t = const.tile([128, 2 * 128], F32R, tag="s1t")
        nc.sync.dma_start(s1t[:].rearrange("p (n c) -> p n c", n=2),
                          s1_d.rearrange("n p c -> p n c"))
        s1p = const.tile([128, 2 * 64], F32R, tag="s1p")
        nc.sync.dma_start(s1p[:].rearrange("p (n c) -> p n c", n=2),
                          s1p_d.rearrange("n p c -> p n c"))
        tw = const.tile([128, 6144], BF16, tag="tw")
        nc.sync.dma_start(tw[:], tw_d)
        id16 = const.tile([128, 128], BF16, tag="id16")
        nc.sync.dma_start(id16[:], id16_d)

        sidx = tabs["st128_idx"]
        M = lambda name: st[:, sidx[name] * 128:(sidx[name] + 1) * 128]
        ident = M("ident")
        ident16 = id16[:]
        s1_re, s1_im = s1t[:, 0:128], s1t[:, 128:256]
        s1p_re, s1p_im = s1p[:, 0:64], s1p[:, 64:128]
        tw1_re, tw1_im = tw[:, 0:1024], tw[:, 1024:2048]
        tw1p_re, tw1p_im = tw[:, 2048:4096], tw[:, 4096:6144]

        MMI = nc.tensor.matmul

        def ecopy(eng, out, in_):
            if eng is nc.scalar:
                nc.scalar.copy(out, in_)
            else:
                eng.tensor_copy(out, in_)

        def pair(dt=F32):
            pr = psum.tile([128, 512], dt, tag="pr")
            pi = psum.tile([128, 512], dt, tag="pi")
            return pr, pi

        def pair16():
            pr = psum.tile([128, 512], BF16, tag="tp")
            pi = psum.tile([128, 512], BF16, tag="tpi")
            return pr, pi

        def cmul_ev(pre, pim, twre, twim, ore, oim):
            """(ore,oim) = (pre+i*pim) * (twre+i*twim), 512-wide.
            DVE: 4 PSUM-source muls; GPS: 2 SBUF-source adds."""
            m1 = tmp.tile([128, 512], F32, tag="m1")
            m2 = tmp.tile([128, 512], F32, tag="m2")
            m3 = tmp.tile([128, 512], F32, tag="m3")
            m4 = tmp.tile([128, 512], F32, tag="m4")
            nc.vector.tensor_tensor(m1[:], pre, twre, MUL)
            nc.vector.tensor_tensor(m2[:], pim, twim, MUL)
            nc.vector.tensor_tensor(m3[:], pre, twim, MUL)
            nc.vector.tensor_tensor(m4[:], pim, twre, MUL)
            nc.gpsimd.tensor_tensor(ore, m1[:], m2[:], SUB)
            nc.gpsimd.tensor_tensor(oim, m3[:], m4[:], ADD)

        for blk in range(n_blocks):
            ch0 = 2 * blk
            # ---- load xh [128, 2048] f32r (rows 0-63 bf16(x), 64-127 h) ----
            xh = data.tile([128, 2048], F32R, tag="xh")
            nc.sync.dma_start(
                xh[:].rearrange("p (n m) -> p n m", n=2),
                xh_d[ch0:ch0 + 2].rearrange("n p m -> p n m"))

            # ---- S1 + EV1(TW1): B1 [c x (ch*1024 + a*8 + b)] ----
            b1re = pool16.tile([128, 2048], BF16, tag="t16re")
            b1im = pool16.tile([128, 2048], BF16, tag="t16im")
            for ck in range(4):            # 512-chunks of (ch*1024 + m)
                cs = slice(ck * 512, (ck + 1) * 512)
                pr, pi = pair()
                MMI(pr[:], s1_re, xh[:, cs], start=True, stop=True)
                MMI(pi[:], s1_im, xh[:, cs], start=True, stop=True)
                ts_ = slice((ck % 2) * 512, (ck % 2) * 512 + 512)
                cmul_ev(pr[:], pi[:], tw1_re[:, ts_], tw1_im[:, ts_],
                        b1re[:, cs], b1im[:, cs])

            # ---- TR1: B2 [a x (b*256 + ch*128 + c)] ----
            b2re = stageB.tile([128, 2048], F32R, tag="pBre")
            b2im = stageB.tile([128, 2048], F32R, tag="pBim")
            b1v_re = b1re[:].rearrange("p (ch a b) -> p ch a b", ch=2, a=128, b=8)
            b1v_im = b1im[:].rearrange("p (ch a b) -> p ch a b", ch=2, a=128, b=8)
            b2v_re = b2re[:].rearrange("p (b ch c) -> p b ch c", b=8, ch=2, c=128)
            b2v_im = b2im[:].rearrange("p (b ch c) -> p b ch c", b=8, ch=2, c=128)
            for ch in range(2):
                for hb in range(2):        # b half: 4 transposes per psum tile
                    pr, pi = pair16()
                    for j in range(4):
                        b = hb * 4 + j
                        s = slice(j * 128, (j + 1) * 128)
                        nc.tensor.transpose(pr[:, s], b1v_re[:, ch, :, b], ident16)
                        nc.tensor.transpose(pi[:, s], b1v_im[:, ch, :, b], ident16)
                    eng = nc.scalar if ch == 0 else nc.vector
                    for ps, ov in ((pr, b2v_re), (pi, b2v_im)):
                        ecopy(eng,
                              ov[:, hb * 4:(hb + 1) * 4, ch, :],
                              ps[:].rearrange("p (j c) -> p j c", j=4))

            # ---- S2: B3 [d x (ch*1024 + c*8 + b)] ----
            b3re = stage.tile([128, 2048], F32R, tag="pAre")
            b3im = stage.tile([128, 2048], F32R, tag="pAim")
            b3v_re = b3re[:].rearrange("p (ch c b) -> p ch c b", ch=2, c=128, b=8)
            b3v_im = b3im[:].rearrange("p (ch c b) -> p ch c b", ch=2, c=128, b=8)
            for hb in range(4):            # 2 b per psum pair
                pr, pi = pair()
                for j in range(2):
                    b = hb * 2 + j
                    s = slice(j * 256, (j + 1) * 256)
                    rs = slice(b * 256, (b + 1) * 256)
                    MMI(pr[:, s], M(f"s2_re{b}"), b2re[:, rs], start=True, stop=False)
                    MMI(pi[:, s], M(f"s2_re{b}"), b2im[:, rs], start=True, stop=False)
                    MMI(pr[:, s], M(f"s2_imn{b}"), b2im[:, rs], start=False, stop=True)
                    MMI(pi[:, s], M(f"s2_im{b}"), b2re[:, rs], start=False, stop=True)
                eng = nc.scalar if hb % 2 == 0 else nc.vector
                for ps, ov in ((pr, b3v_re), (pi, b3v_im)):
                    iv = ps[:].rearrange("p (j ch c) -> p j ch c", j=2, ch=2)
                    for ch in range(2):
                        ecopy(eng,
                              ov[:, ch, :, hb * 2:(hb + 1) * 2]
                              .rearrange("p c j -> p j c"),
                              iv[:, :, ch, :])

            # ---- TR2: B4 [(c16*8+b) x (ch*1024 + chi*128 + d)] ----
            b4re = stageB.tile([128, 2048], F32R, tag="pBre")
            b4im = stageB.tile([128, 2048], F32R, tag="pBim")
            for ch in range(2):
                for hc in range(2):        # chi half
                    pr, pi = pair(F32R)
                    for j in range(4):
                        chi = hc * 4 + j
                        s = slice(j * 128, (j + 1) * 128)
                        src = slice(ch * 1024 + chi * 128, ch * 1024 + (chi + 1) * 128)
                        nc.tensor.transpose(pr[:, s], b3re[:, src], ident)
                        nc.tensor.transpose(pi[:, s], b3im[:, src], ident)
                    ds = slice(ch * 1024 + hc * 512, ch * 1024 + (hc + 1) * 512)
                    eng = nc.scalar if ch == 0 else nc.vector
                    ecopy(eng, b4re[:, ds], pr[:])
                    ecopy(eng, b4im[:, ds], pi[:])

            # ---- S3 + EV5(square): B5 = (Sre, Sim/2) ----
            b5re = stage.tile([128, 2048], F32R, tag="pAre")
            b5im = stage.tile([128, 2048], F32R, tag="pAim")
            for ckp in range(2):
                c0 = slice((2 * ckp) * 512, (2 * ckp + 1) * 512)
                c1 = slice((2 * ckp + 1) * 512, (2 * ckp + 2) * 512)
                pr0, pi0 = pair()
                pr1, pi1 = pair()
                MMI(pr0[:], M("s3_re"), b4re[:, c0], start=True, stop=False)
                MMI(pi0[:], M("s3_re"), b4im[:, c0], start=True, stop=False)
                MMI(pr1[:], M("s3_re"), b4re[:, c1], start=True, stop=False)
                MMI(pi1[:], M("s3_re"), b4im[:, c1], start=True, stop=False)
                MMI(pr0[:], M("s3_imn"), b4im[:, c0], start=False, stop=True)
                MMI(pr1[:], M("s3_imn"), b4im[:, c1], start=False, stop=True)
                MMI(pi0[:], M("s3_im"), b4re[:, c0], start=False, stop=True)
                MMI(pi1[:], M("s3_im"), b4re[:, c1], start=False, stop=True)
                for cs, pr, pi in ((c0, pr0, pi0), (c1, pr1, pi1)):
                    sqre = tmp.tile([128, 512], F32, tag="m1")
                    sqim = tmp.tile([128, 512], F32, tag="m2")
                    zim = tmp.tile([128, 512], F32, tag="m3")
                    nc.scalar.activation(sqre[:], pr[:],
                                         mybir.ActivationFunctionType.Square)
                    nc.scalar.activation(sqim[:], pi[:],
                                         mybir.ActivationFunctionType.Square)
                    nc.scalar.copy(zim[:], pi[:])
                    nc.gpsimd.tensor_tensor(b5re[:, cs], sqre[:], sqim[:], SUB)
                    nc.vector.tensor_tensor(b5im[:, cs], pr[:], zim[:], MUL)

            # ---- S3': B6 [(c16*8+bp) x cols] ----
            b6re = pool16.tile([128, 2048], BF16, tag="t16re")
            b6im = pool16.tile([128, 2048], BF16, tag="t16im")
            for ckp in range(2):
                c0 = slice((2 * ckp) * 512, (2 * ckp + 1) * 512)
                c1 = slice((2 * ckp + 1) * 512, (2 * ckp + 2) * 512)
                pr0, pi0 = pair()
                pr1, pi1 = pair()
                MMI(pr0[:], M("s3p_re"), b5re[:, c0], start=True, stop=False)
                MMI(pr1[:], M("s3p_re"), b5re[:, c1], start=True, stop=False)
                MMI(pi0[:], M("s3p_im"), b5re[:, c0], start=True, stop=False)
                MMI(pi1[:], M("s3p_im"), b5re[:, c1], start=True, stop=False)
                MMI(pr0[:], M("s3p_imn2"), b5im[:, c0], start=False, stop=True)
                MMI(pr1[:], M("s3p_imn2"), b5im[:, c1], start=False, stop=True)
                MMI(pi0[:], M("s3p_re2"), b5im[:, c0], start=False, stop=True)
                MMI(pi1[:], M("s3p_re2"), b5im[:, c1], start=False, stop=True)
                for j, (cs, pr, pi) in enumerate(((c0, pr0, pi0), (c1, pr1, pi1))):
                    eng = nc.scalar if j == 0 else nc.vector
                    ecopy(eng, b6re[:, cs], pr[:])
                    ecopy(eng, b6im[:, cs], pi[:])

            # ---- TR3: B7 [d x (bp*256 + ch*128 + c)] ----
            b7re = stage.tile([128, 2048], F32R, tag="pAre")
            b7im = stage.tile([128, 2048], F32R, tag="pAim")
            b7v_re = b7re[:].rearrange("p (bp ch chi c16) -> p bp ch chi c16",
                                       bp=8, ch=2, chi=8, c16=16)
            b7v_im = b7im[:].rearrange("p (bp ch chi c16) -> p bp ch chi c16",
                                       bp=8, ch=2, chi=8, c16=16)
            for ch in range(2):
                for hc in range(2):
                    pr, pi = pair16()
                    for j in range(4):
                        chi = hc * 4 + j
                        s = slice(j * 128, (j + 1) * 128)
                        src = slice(ch * 1024 + chi * 128, ch * 1024 + (chi + 1) * 128)
                        nc.tensor.transpose(pr[:, s], b6re[:, src], ident16)
                        nc.tensor.transpose(pi[:, s], b6im[:, src], ident16)
                    eng = nc.vector if ch == 0 else nc.scalar
                    for ps, ov in ((pr, b7v_re), (pi, b7v_im)):
                        iv = ps[:].rearrange("p (chi c16 bp) -> p chi c16 bp",
                                             chi=4, c16=16, bp=8)
                        ecopy(eng,
                              ov[:, :, ch, hc * 4:(hc + 1) * 4, :]
                              .rearrange("p bp chi c16 -> p chi c16 bp"), iv[:])

            # ---- S2' + EV8(TW1'): B8 [a x (bp*256 + ch*128 + c)] ----
            b8re = pool16.tile([128, 2048], BF16, tag="t16re")
            b8im = pool16.tile([128, 2048], BF16, tag="t16im")
            for hb in range(4):
                pr, pi = pair()
                for j in range(2):
                    b = hb * 2 + j
                    s = slice(j * 256, (j + 1) * 256)
                    rs = slice(b * 256, (b + 1) * 256)
                    MMI(pr[:, s], M(f"s2p_re{b}"), b7re[:, rs], start=True, stop=False)
                    MMI(pi[:, s], M(f"s2p_re{b}"), b7im[:, rs], start=True, stop=False)
                    MMI(pr[:, s], M(f"s2p_imn{b}"), b7im[:, rs], start=False, stop=True)
                    MMI(pi[:, s], M(f"s2p_im{b}"), b7re[:, rs], start=False, stop=True)
                cs = slice(hb * 512, (hb + 1) * 512)
                cmul_ev(pr[:], pi[:], tw1p_re[:, cs], tw1p_im[:, cs],
                        b8re[:, cs], b8im[:, cs])

            # ---- TR4: B9 [c x (ch*1024 + a*8 + bp)] ----
            b9re = stage.tile([128, 2048], F32R, tag="pAre")
            b9im = stage.tile([128, 2048], F32R, tag="pAim")
            b8v_re = b8re[:].rearrange("p (bp ch c) -> p bp ch c", bp=8, ch=2, c=128)
            b8v_im = b8im[:].rearrange("p (bp ch c) -> p bp ch c", bp=8, ch=2, c=128)
            b9v_re = b9re[:].rearrange("p (ch a bp) -> p ch a bp", ch=2, a=128, bp=8)
            b9v_im = b9im[:].rearrange("p (ch a bp) -> p ch a bp", ch=2, a=128, bp=8)
            for ch in range(2):
                for hb in range(2):
                    pr, pi = pair16()
                    for j in range(4):
                        bp = hb * 4 + j
                        s = slice(j * 128, (j + 1) * 128)
                        nc.tensor.transpose(pr[:, s], b8v_re[:, bp, ch, :], ident16)
                        nc.tensor.transpose(pi[:, s], b8v_im[:, bp, ch, :], ident16)
                    eng = nc.scalar if ch == 0 else nc.vector
                    for ps, ov in ((pr, b9v_re), (pi, b9v_im)):
                        ecopy(eng,
                              ov[:, ch, :, hb * 4:(hb + 1) * 4]
                              .rearrange("p a j -> p j a"),
                              ps[:].rearrange("p (j a) -> p j a", j=4))

            # ---- S1' + store (imag plane only) ----
            for ch in range(2):
                for q in range(2):
                    p10 = psum.tile([64, 512], F32, tag="pr")
                    rs = slice(ch * 1024 + q * 512, ch * 1024 + (q + 1) * 512)
                    MMI(p10[:], s1p_im, b9re[:, rs], start=True, stop=False)
                    MMI(p10[:], s1p_re, b9im[:, rs], start=False, stop=True)
                    yt = data.tile([64, 512], F32, tag="yt")
                    nc.scalar.copy(yt[:], p10[:])
                    nc.sync.dma_start(
                        y_d[ch0 + ch].rearrange("u (q m) -> u q m", q=2)[:, q, :],
                        yt[:])

    nc.compile()
    return nc, tabs


_CACHE = {}


def _get(n_blocks=NBLK):
    key = n_blocks
    if key not in _CACHE:
        _CACHE[key] = _build(n_blocks)
    return _CACHE[key]


def _in_maps(x, h, tabs):
    xf = np.ascontiguousarray(x, np.float32).reshape(Bsz * Csz, 64, 1024)
    hf = np.ascontiguousarray(h, np.float32).reshape(Bsz * Csz, 64, 1024)
    xq = xf.astype(ml_dtypes.bfloat16).astype(np.float32)
    xh = np.concatenate([xq, hf], axis=1)                  # [BC, 128, 1024] f32
    maps = []
    for i in range(NCORES):
        sl = slice(i * CPC, (i + 1) * CPC)
        maps.append({
            "xh_in": xh[sl],
            "st_in": tabs["st128"],
            "s1_in": tabs["s1"],
            "s1p_in": tabs["s1p"],
            "tw_in": tabs["tw"],
            "id16_in": np.eye(128, dtype=ml_dtypes.bfloat16),
        })
    return maps


def kernel(x, h):
    nc, tabs = _get()
    maps = _in_maps(x, h, tabs)
    res = run_bass_kernel_spmd(nc, maps, core_ids=list(range(NCORES)))
    y = np.concatenate([r["y_out"].reshape(CPC, 65536) for r in res.results])
    return y.reshape(Bsz, Csz, T).astype(np.float32)


# revision 13
# speedup vs baseline: 1.1342x; 1.1342x over previous
"""Trainium2 Bass kernel for causal FFT convolution (nn_CausalConvolution).

y = irfft(rfft(bf16(x), 2T) * rfft(h, 2T))[..., :T],  x,h: (8, 64, 65536) fp32.

Identity used: with z = bf16(x) + i*h,  y = Im(iFFT(FFT_2T(z)^2)) / 2.
One complex forward + one complex inverse FFT per channel, N = 131072,
decomposed as radix (128, 128, 8) matmul stages on the PE with PE-transposes
between stages. 512 channels sharded 64-per-core across 8 NeuronCores.

v3 (from measured v1/v2 profiles): f32r datapath (PSUM-source elementwise ops
dodge the SBUF-source errata and cast penalties), x/h stacked into a single
128-partition f32r rhs for stage 1 (halves its matmuls), GpSimd absorbs the
SBUF-source adds (it was idle), transpose evacuations split scalar/vector.

Self-contained: shapes/sharding hardcoded; tables computed with numpy here.
"""
import numpy as np
import ml_dtypes
from contextlib import ExitStack

import concourse.bass as bass
import concourse.bacc as bacc
import concourse.tile as tile
import concourse.mybir as mybir
from concourse.bass_utils import run_bass_kernel_spmd

F32 = mybir.dt.float32
F32R = mybir.dt.float32r
BF16 = mybir.dt.bfloat16
MUL = mybir.AluOpType.mult
ADD = mybir.AluOpType.add
SUB = mybir.AluOpType.subtract

Bsz, Csz, T = 8, 64, 65536
NFFT = 2 * T
NCORES = 8
CPC = (Bsz * Csz) // NCORES          # 64 channels per core
NBLK = CPC // 2                      # 2 channels per block

_Wc = lambda M, E: np.exp(-2j * np.pi * E / M)


def _gen_tables():
    F128 = _Wc(128, np.outer(np.arange(128), np.arange(128)))
    F8 = _Wc(8, np.outer(np.arange(8), np.arange(8)))
    W1024bd = _Wc(1024, np.outer(np.arange(8), np.arange(128)))     # [b, d]
    TW1_cab = _Wc(NFFT, (8 * np.arange(128)[None, :, None]
                         + np.arange(8)[None, None, :])
                  * np.arange(128)[:, None, None])                  # [c, a, b]

    f32 = lambda v: np.ascontiguousarray(v, dtype=np.float32)
    t = {}
    # ---- S1 stacked stationaries: rhs rows 0-63 = bf16(x), 64-127 = h ----
    s1re = np.vstack([F128[:64].real, -F128[:64].imag])             # [128, 128]
    s1im = np.vstack([F128[:64].imag, F128[:64].real])
    t["s1"] = np.stack([f32(s1re), f32(s1im)])                      # [2,128,128]

    # ---- packed [n,128,128] f32r stationaries ----
    mats = []
    idx = {}

    def put(name, m):
        idx[name] = len(mats)
        mats.append(f32(m))

    S2 = F128[None, :, :] * W1024bd[:, None, :]                     # [b, a, d]
    for b in range(8):
        put(f"s2_re{b}", S2[b].real)
        put(f"s2_im{b}", S2[b].imag)
        put(f"s2_imn{b}", -S2[b].imag)
    S3 = np.zeros((128, 128), np.complex128)
    for b in range(8):
        for e in range(8):
            for c16 in range(16):
                S3[c16 * 8 + b, c16 * 8 + e] = F8[b, e]
    put("s3_re", S3.real)
    put("s3_im", S3.imag)
    put("s3_imn", -S3.imag)
    S3p = np.zeros((128, 128), np.complex128)
    for e in range(8):
        for bp in range(8):
            for c16 in range(16):
                S3p[c16 * 8 + e, c16 * 8 + bp] = np.conj(F8[bp, e])
    put("s3p_re", S3p.real)          # multiplies rhs_re -> psum_re
    put("s3p_im", S3p.imag)          # rhs_re -> psum_im
    put("s3p_imn2", -2 * S3p.imag)   # rhs_im -> psum_re (x2: Sim stored halved)
    put("s3p_re2", 2 * S3p.real)     # rhs_im -> psum_im
    S2p = np.conj(S2).transpose(0, 2, 1)                            # [bp, d, a]
    for b in range(8):
        put(f"s2p_re{b}", S2p[b].real)
        put(f"s2p_im{b}", S2p[b].imag)
        put(f"s2p_imn{b}", -S2p[b].imag)
    put("ident", np.eye(128))
    t["st128"] = np.stack(mats)                                     # [n,128,128]
    t["st128_idx"] = idx

    # ---- S1' (contract c, produce u<64, imag plane only, scale 1/(2N)) ----
    S1p = np.conj(F128).T[:, :64] / (2.0 * NFFT)                    # [c, u]
    t["s1p"] = np.stack([f32(S1p.real), f32(S1p.imag)])             # [2,128,64]

    # ---- twiddle tables ----
    m_ord = TW1_cab.reshape(128, 1024)
    tw1p = np.zeros((128, 2048), np.complex128)                     # [a, bp*256+ch*128+c]
    for bp in range(8):
        for ch in range(2):
            sl = slice(bp * 256 + ch * 128, bp * 256 + ch * 128 + 128)
            tw1p[:, sl] = np.conj(TW1_cab[:, :, bp]).T
    bf = lambda v: np.ascontiguousarray(v, np.float32).astype(ml_dtypes.bfloat16)
    t["tw"] = np.concatenate(
        [bf(m_ord.real), bf(m_ord.imag),
         bf(tw1p.real), bf(tw1p.imag)], axis=1)                     # [128, 6144]
    return t


def _build(n_blocks=NBLK):
    tabs = _gen_tables()
    nc = bacc.Bacc("TRN2", target_bir_lowering=False, debug=False)

    xh_d = nc.dram_tensor("xh_in", [CPC, 128, 1024], F32R, kind="ExternalInput").ap()
    nst = tabs["st128"].shape[0]
    st_d = nc.dram_tensor("st_in", [nst, 128, 128], F32R, kind="ExternalInput").ap()
    s1_d = nc.dram_tensor("s1_in", [2, 128, 128], F32R, kind="ExternalInput").ap()
    s1p_d = nc.dram_tensor("s1p_in", [2, 128, 64], F32R, kind="ExternalInput").ap()
    tw_d = nc.dram_tensor("tw_in", [128, 6144], BF16, kind="ExternalInput").ap()
    id16_d = nc.dram_tensor("id16_in", [128, 128], BF16, kind="ExternalInput").ap()
    y_d = nc.dram_tensor("y_out", [CPC, 64, 1024], F32, kind="ExternalOutput").ap()

    with tile.TileContext(nc) as tc, ExitStack() as ctx:
        const = ctx.enter_context(tc.tile_pool(name="const", bufs=1))
        data = ctx.enter_context(tc.tile_pool(name="io", bufs=2))
        stage = ctx.enter_context(tc.tile_pool(name="stage", bufs=4))
        stageB = ctx.enter_context(tc.tile_pool(name="stageB", bufs=2))
        tmp = ctx.enter_context(tc.tile_pool(name="tmp", bufs=2))
        pool16 = ctx.enter_context(tc.tile_pool(name="p16", bufs=3))
        psum = ctx.enter_context(tc.tile_pool(name="psum", bufs=4, space="PSUM"))

        # ---- load constant tables once ----
        st = const.tile([128, nst * 128], F32R, tag="st")
        nc.sync.dma_start(
            st[:].rearrange("p (n c) -> p n c", n=nst),
            st_d.rearrange("n p c -> p n c"))
        s1t = const.tile([128, 2 * 128], F32R, tag="s1t")
        nc.sync.dma_start(s1t[:].rearrange("p (n c) -> p n c", n=2),
                          s1_d.rearrange("n p c -> p n c"))
        s1p = const.tile([128, 2 * 64], F32R, tag="s1p")
        nc.sync.dma_start(s1p[:].rearrange("p (n c) -> p n c", n=2),
                          s1p_d.rearrange("n p c -> p n c"))
        tw = const.tile([128, 6144], BF16, tag="tw")
        nc.sync.dma_start(tw[:], tw_d)
        id16 = const.tile([128, 128], BF16, tag="id16")
        nc.sync.dma_start(id16[:], id16_d)

        sidx = tabs["st128_idx"]
        M = lambda name: st[:, sidx[name] * 128:(sidx[name] + 1) * 128]
        ident = M("ident")
        ident16 = id16[:]
        s1_re, s1_im = s1t[:, 0:128], s1t[:, 128:256]
        s1p_re, s1p_im = s1p[:, 0:64], s1p[:, 64:128]
        tw1_re, tw1_im = tw[:, 0:1024], tw[:, 1024:2048]
        tw1p_re, tw1p_im = tw[:, 2048:4096], tw[:, 4096:6144]

        MMI = nc.tensor.matmul

        def ecopy(eng, out, in_):
            if eng is nc.scalar:
                nc.scalar.copy(out, in_)
            else:
                eng.tensor_copy(out, in_)

        def pair(dt=F32):
            pr = psum.tile([128, 512], dt, tag="pr")
            pi = psum.tile([128, 512], dt, tag="pi")
            return pr, pi

        def pair16():
            pr = psum.tile([128, 512], BF16, tag="pr")
            pi = psum.tile([128, 512], BF16, tag="pi")
            return pr, pi

        def cmul_ev(pre, pim, twre, twim, ore, oim):
            """(ore,oim) = (pre+i*pim) * (twre+i*twim), 512-wide.
            DVE: 4 PSUM-source muls; GPS: 2 SBUF-source adds."""
            m1 = tmp.tile([128, 512], F32, tag="m1")
            m2 = tmp.tile([128, 512], F32, tag="m2")
            m3 = tmp.tile([128, 512], F32, tag="m3")
            m4 = tmp.tile([128, 512], F32, tag="m4")
            nc.vector.tensor_tensor(m1[:], pre, twre, MUL)
            nc.vector.tensor_tensor(m2[:], pim, twim, MUL)
            nc.vector.tensor_tensor(m3[:], pre, twim, MUL)
            nc.vector.tensor_tensor(m4[:], pim, twre, MUL)
            nc.gpsimd.tensor_tensor(ore, m1[:], m2[:], SUB)
            nc.gpsimd.tensor_tensor(oim, m3[:], m4[:], ADD)

        for blk in range(n_blocks):
            ch0 = 2 * blk
            # ---- load xh [128, 2048] f32r (rows 0-63 bf16(x), 64-127 h) ----
            xh = data.tile([128, 2048], F32R, tag="xh")
            nc.sync.dma_start(
                xh[:].rearrange("p (n m) -> p n m", n=2),
                xh_d[ch0:ch0 + 2].rearrange("n p m -> p n m"))

            # ---- S1 + EV1(TW1): B1 [c x (ch*1024 + a*8 + b)] ----
            b1re = pool16.tile([128, 2048], BF16, tag="t16re")
            b1im = pool16.tile([128, 2048], BF16, tag="t16im")
            for ck in range(4):            # 512-chunks of (ch*1024 + m)
                cs = slice(ck * 512, (ck + 1) * 512)
                pr, pi = pair()
                MMI(pr[:], s1_re, xh[:, cs], start=True, stop=True)
                MMI(pi[:], s1_im, xh[:, cs], start=True, stop=True)
                ts_ = slice((ck % 2) * 512, (ck % 2) * 512 + 512)
                cmul_ev(pr[:], pi[:], tw1_re[:, ts_], tw1_im[:, ts_],
                        b1re[:, cs], b1im[:, cs])

            # ---- TR1: B2 [a x (b*256 + ch*128 + c)] ----
            b2re = stageB.tile([128, 2048], F32R, tag="pBre")
            b2im = stageB.tile([128, 2048], F32R, tag="pBim")
            b1v_re = b1re[:].rearrange("p (ch a b) -> p ch a b", ch=2, a=128, b=8)
            b1v_im = b1im[:].rearrange("p (ch a b) -> p ch a b", ch=2, a=128, b=8)
            b2v_re = b2re[:].rearrange("p (b ch c) -> p b ch c", b=8, ch=2, c=128)
            b2v_im = b2im[:].rearrange("p (b ch c) -> p b ch c", b=8, ch=2, c=128)
            for ch in range(2):
                for hb in range(2):        # b half: 4 transposes per psum tile
                    pr, pi = pair16()
                    for j in range(4):
                        b = hb * 4 + j
                        s = slice(j * 128, (j + 1) * 128)
                        nc.tensor.transpose(pr[:, s], b1v_re[:, ch, :, b], ident16)
                        nc.tensor.transpose(pi[:, s], b1v_im[:, ch, :, b], ident16)
                    eng = nc.scalar if ch == 0 else nc.vector
                    for ps, ov in ((pr, b2v_re), (pi, b2v_im)):
                        ecopy(eng,
                              ov[:, hb * 4:(hb + 1) * 4, ch, :],
                              ps[:].rearrange("p (j c) -> p j c", j=4))

            # ---- S2: B3 [d x (ch*1024 + c*8 + b)] ----
            b3re = stage.tile([128, 2048], F32R, tag="pAre")
            b3im = stage.tile([128, 2048], F32R, tag="pAim")
            b3v_re = b3re[:].rearrange("p (ch c b) -> p ch c b", ch=2, c=128, b=8)
            b3v_im = b3im[:].rearrange("p (ch c b) -> p ch c b", ch=2, c=128, b=8)
            for hb in range(4):            # 2 b per psum pair
                pr, pi = pair()
                for j in range(2):
                    b = hb * 2 + j
                    s = slice(j * 256, (j + 1) * 256)
                    rs = slice(b * 256, (b + 1) * 256)
                    MMI(pr[:, s], M(f"s2_re{b}"), b2re[:, rs], start=True, stop=False)
                    MMI(pi[:, s], M(f"s2_re{b}"), b2im[:, rs], start=True, stop=False)
                    MMI(pr[:, s], M(f"s2_imn{b}"), b2im[:, rs], start=False, stop=True)
                    MMI(pi[:, s], M(f"s2_im{b}"), b2re[:, rs], start=False, stop=True)
                eng = nc.scalar if hb % 2 == 0 else nc.vector
                for ps, ov in ((pr, b3v_re), (pi, b3v_im)):
                    iv = ps[:].rearrange("p (j ch c) -> p j ch c", j=2, ch=2)
                    for ch in range(2):
                        ecopy(eng,
                              ov[:, ch, :, hb * 2:(hb + 1) * 2]
                              .rearrange("p c j -> p j c"),
                              iv[:, :, ch, :])

            # ---- TR2: B4 [(c16*8+b) x (ch*1024 + chi*128 + d)] ----
            b4re = stageB.tile([128, 2048], F32R, tag="pBre")
            b4im = stageB.tile([128, 2048], F32R, tag="pBim")
            for ch in range(2):
                for hc in range(2):        # chi half
                    pr, pi = pair(F32R)
                    for j in range(4):
                        chi = hc * 4 + j
                        s = slice(j * 128, (j + 1) * 128)
                        src = slice(ch * 1024 + chi * 128, ch * 1024 + (chi + 1) * 128)
                        nc.tensor.transpose(pr[:, s], b3re[:, src], ident)
                        nc.tensor.transpose(pi[:, s], b3im[:, src], ident)
                    ds = slice(ch * 1024 + hc * 512, ch * 1024 + (hc + 1) * 512)
                    eng = nc.scalar if ch == 0 else nc.vector
                    ecopy(eng, b4re[:, ds], pr[:])
                    ecopy(eng, b4im[:, ds], pi[:])

            # ---- S3 + EV5(square): B5 = (Sre, Sim/2) ----
            b5re = stage.tile([128, 2048], F32R, tag="pAre")
            b5im = stage.tile([128, 2048], F32R, tag="pAim")
            for ckp in range(2):
                c0 = slice((2 * ckp) * 512, (2 * ckp + 1) * 512)
                c1 = slice((2 * ckp + 1) * 512, (2 * ckp + 2) * 512)
                pr0, pi0 = pair()
                pr1, pi1 = pair()
                MMI(pr0[:], M("s3_re"), b4re[:, c0], start=True, stop=False)
                MMI(pi0[:], M("s3_re"), b4im[:, c0], start=True, stop=False)
                MMI(pr1[:], M("s3_re"), b4re[:, c1], start=True, stop=False)
                MMI(pi1[:], M("s3_re"), b4im[:, c1], start=True, stop=False)
                MMI(pr0[:], M("s3_imn"), b4im[:, c0], start=False, stop=True)
                MMI(pr1[:], M("s3_imn"), b4im[:, c1], start=False, stop=True)
                MMI(pi0[:], M("s3_im"), b4re[:, c0], start=False, stop=True)
                MMI(pi1[:], M("s3_im"), b4re[:, c1], start=False, stop=True)
                for cs, pr, pi in ((c0, pr0, pi0), (c1, pr1, pi1)):
                    sqre = tmp.tile([128, 512], F32, tag="m1")
                    sqim = tmp.tile([128, 512], F32, tag="m2")
                    zim = tmp.tile([128, 512], F32, tag="m3")
                    nc.scalar.activation(sqre[:], pr[:],
                                         mybir.ActivationFunctionType.Square)
                    nc.scalar.activation(sqim[:], pi[:],
                                         mybir.ActivationFunctionType.Square)
                    nc.scalar.copy(zim[:], pi[:])
                    nc.gpsimd.tensor_tensor(b5re[:, cs], sqre[:], sqim[:], SUB)
                    nc.vector.tensor_tensor(b5im[:, cs], pr[:], zim[:], MUL)

            # ---- S3': B6 [(c16*8+bp) x cols] ----
            b6re = pool16.tile([128, 2048], BF16, tag="t16re")
            b6im = pool16.tile([128, 2048], BF16, tag="t16im")
            for ckp in range(2):
                c0 = slice((2 * ckp) * 512, (2 * ckp + 1) * 512)
                c1 = slice((2 * ckp + 1) * 512, (2 * ckp + 2) * 512)
                pr0, pi0 = pair()
                pr1, pi1 = pair()
                MMI(pr0[:], M("s3p_re"), b5re[:, c0], start=True, stop=False)
                MMI(pr1[:], M("s3p_re"), b5re[:, c1], start=True, stop=False)
                MMI(pi0[:], M("s3p_im"), b5re[:, c0], start=True, stop=False)
                MMI(pi1[:], M("s3p_im"), b5re[:, c1], start=True, stop=False)
                MMI(pr0[:], M("s3p_imn2"), b5im[:, c0], start=False, stop=True)
                MMI(pr1[:], M("s3p_imn2"), b5im[:, c1], start=False, stop=True)
                MMI(pi0[:], M("s3p_re2"), b5im[:, c0], start=False, stop=True)
                MMI(pi1[:], M("s3p_re2"), b5im[:, c1], start=False, stop=True)
                for j, (cs, pr, pi) in enumerate(((c0, pr0, pi0), (c1, pr1, pi1))):
                    eng = nc.scalar if j == 0 else nc.vector
                    ecopy(eng, b6re[:, cs], pr[:])
                    ecopy(eng, b6im[:, cs], pi[:])

            # ---- TR3: B7 [d x (bp*256 + ch*128 + c)] ----
            b7re = stage.tile([128, 2048], F32R, tag="pAre")
            b7im = stage.tile([128, 2048], F32R, tag="pAim")
            b7v_re = b7re[:].rearrange("p (bp ch chi c16) -> p bp ch chi c16",
                                       bp=8, ch=2, chi=8, c16=16)
            b7v_im = b7im[:].rearrange("p (bp ch chi c16) -> p bp ch chi c16",
                                       bp=8, ch=2, chi=8, c16=16)
            for ch in range(2):
                for hc in range(2):
                    pr, pi = pair16()
                    for j in range(4):
                        chi = hc * 4 + j
                        s = slice(j * 128, (j + 1) * 128)
                        src = slice(ch * 1024 + chi * 128, ch * 1024 + (chi + 1) * 128)
                        nc.tensor.transpose(pr[:, s], b6re[:, src], ident16)
                        nc.tensor.transpose(pi[:, s], b6im[:, src], ident16)
                    eng = nc.vector if ch == 0 else nc.scalar
                    for ps, ov in ((pr, b7v_re), (pi, b7v_im)):
                        iv = ps[:].rearrange("p (chi c16 bp) -> p chi c16 bp",
                                             chi=4, c16=16, bp=8)
                        ecopy(eng,
                              ov[:, :, ch, hc * 4:(hc + 1) * 4, :]
                              .rearrange("p bp chi c16 -> p chi c16 bp"), iv[:])

            # ---- S2' + EV8(TW1'): B8 [a x (bp*256 + ch*128 + c)] ----
            b8re = pool16.tile([128, 2048], BF16, tag="t16re")
            b8im = pool16.tile([128, 2048], BF16, tag="t16im")
            for hb in range(4):
                pr, pi = pair()
                for j in range(2):
                    b = hb * 2 + j
                    s = slice(j * 256, (j + 1) * 256)
                    rs = slice(b * 256, (b + 1) * 256)
                    MMI(pr[:, s], M(f"s2p_re{b}"), b7re[:, rs], start=True, stop=False)
                    MMI(pi[:, s], M(f"s2p_re{b}"), b7im[:, rs], start=True, stop=False)
                    MMI(pr[:, s], M(f"s2p_imn{b}"), b7im[:, rs], start=False, stop=True)
                    MMI(pi[:, s], M(f"s2p_im{b}"), b7re[:, rs], start=False, stop=True)
                cs = slice(hb * 512, (hb + 1) * 512)
                cmul_ev(pr[:], pi[:], tw1p_re[:, cs], tw1p_im[:, cs],
                        b8re[:, cs], b8im[:, cs])

            # ---- TR4: B9 [c x (ch*1024 + a*8 + bp)] ----
            b9re = stage.tile([128, 2048], F32R, tag="pAre")
            b9im = stage.tile([128, 2048], F32R, tag="pAim")
            b8v_re = b8re[:].rearrange("p (bp ch c) -> p bp ch c", bp=8, ch=2, c=128)
            b8v_im = b8im[:].rearrange("p (bp ch c) -> p bp ch c", bp=8, ch=2, c=128)
            b9v_re = b9re[:].rearrange("p (ch a bp) -> p ch a bp", ch=2, a=128, bp=8)
            b9v_im = b9im[:].rearrange("p (ch a bp) -> p ch a bp", ch=2, a=128, bp=8)
            for ch in range(2):
                for hb in range(2):
                    pr, pi = pair16()
                    for j in range(4):
                        bp = hb * 4 + j
                        s = slice(j * 128, (j + 1) * 128)
                        nc.tensor.transpose(pr[:, s], b8v_re[:, bp, ch, :], ident16)
                        nc.tensor.transpose(pi[:, s], b8v_im[:, bp, ch, :], ident16)
                    eng = nc.scalar if ch == 0 else nc.vector
                    for ps, ov in ((pr, b9v_re), (pi, b9v_im)):
                        ecopy(eng,
                              ov[:, ch, :, hb * 4:(hb + 1) * 4]
                              .rearrange("p a j -> p j a"),
                              ps[:].rearrange("p (j a) -> p j a", j=4))

            # ---- S1' + store (imag plane only) ----
            for ch in range(2):
                for q in range(2):
                    p10 = psum.tile([64, 512], F32, tag="pr")
                    rs = slice(ch * 1024 + q * 512, ch * 1024 + (q + 1) * 512)
                    MMI(p10[:], s1p_im, b9re[:, rs], start=True, stop=False)
                    MMI(p10[:], s1p_re, b9im[:, rs], start=False, stop=True)
                    yt = data.tile([64, 512], F32, tag="yt")
                    nc.scalar.copy(yt[:], p10[:])
                    nc.sync.dma_start(
                        y_d[ch0 + ch].rearrange("u (q m) -> u q m", q=2)[:, q, :],
                        yt[:])

    nc.compile()
    return nc, tabs


_CACHE = {}


def _get(n_blocks=NBLK):
    key = n_blocks
    if key not in _CACHE:
        _CACHE[key] = _build(n_blocks)
    return _CACHE[key]


def _in_maps(x, h, tabs):
    xf = np.ascontiguousarray(x, np.float32).reshape(Bsz * Csz, 64, 1024)
    hf = np.ascontiguousarray(h, np.float32).reshape(Bsz * Csz, 64, 1024)
    xq = xf.astype(ml_dtypes.bfloat16).astype(np.float32)
    xh = np.concatenate([xq, hf], axis=1)                  # [BC, 128, 1024] f32
    maps = []
    for i in range(NCORES):
        sl = slice(i * CPC, (i + 1) * CPC)
        maps.append({
            "xh_in": xh[sl],
            "st_in": tabs["st128"],
            "s1_in": tabs["s1"],
            "s1p_in": tabs["s1p"],
            "tw_in": tabs["tw"],
            "id16_in": np.eye(128, dtype=ml_dtypes.bfloat16),
        })
    return maps


def kernel(x, h):
    nc, tabs = _get()
    maps = _in_maps(x, h, tabs)
    res = run_bass_kernel_spmd(nc, maps, core_ids=list(range(NCORES)))
    y = np.concatenate([r["y_out"].reshape(CPC, 65536) for r in res.results])
    return y.reshape(Bsz, Csz, T).astype(np.float32)
